# revision 16
# baseline (speedup 1.0000x reference)
"""MoE routing kernel for Trainium2 (8 NeuronCores, SPMD expert-parallel).

Contract: kernel(**full_inputs) -> full output [B, S, H] float32.

Strategy
--------
- Host: gate + group-topk routing in numpy (bit-identical selection to the
  jax reference), then dispatch: gather each expert's tokens into padded,
  transposed buffers (the "all-to-all by topk_idx" of the sharding hint).
- Device (SPMD over 8 cores): core c holds 2 experts and a 1/8 column
  slice of the shared expert.  Mixed precision by routing rank: tokens for
  which this expert is one of their top-3 choices run the full SwiGLU in
  bf16; rank-3 tokens run gate/up in fp8-e4m3 (DoubleRow, 2x PE rate) with
  a bf16 down; rank-4/5 tokens run everything in fp8.  The fp8 error is
  attenuated by the (smaller) routing weights of the low-rank experts, so
  the end-to-end error stays ~1.8e-2 < 2e-2.
- Host: scale per-expert outputs by routing weights (and the fp8 output
  scale), scatter-add over token indices, add the 8 shared partials.

All matmuls accumulate in fp32 PSUM.  fp8 scales: x*16, w*1024; silu gets
scale=1/16384 on its input; the residual 16384 output factor folds into
the host-side combine weights.
"""

import math
from itertools import combinations

import numpy as np
import ml_dtypes

H = 2048          # hidden size
I = 1408          # intermediate per routed expert
E = 16            # routed experts
G = 4             # groups
TOPK_GROUP = 2
TOP_K = 6
N_SHARED = 2
SCALE_FACTOR = 2.5
SI = I * N_SHARED  # 2816 shared intermediate
N_CORES = 8
EXP_PER_CORE = E // N_CORES  # 2
S_SLICE_RAW = SI // N_CORES  # 352
S_SLICE = 384                # padded to 3*128
P = 128
BF16 = ml_dtypes.bfloat16
F8 = ml_dtypes.float8_e4m3   # TRN FP8_EXP4 (max 240)

XS = 16.0      # fp8 activation scale
WS = 1024.0    # fp8 weight scale
OS = XS * WS   # 16384: scale of fp8-path outputs (divided out on host)

# rank -> class: 'bf' (all bf16), 'gu8' (fp8 gate/up, bf16 down),
# 'f8' (all fp8).  Tune for error budget: sim says this lands ~1.8e-2.
RANK_CLASS = ["bf", "bf", "bf", "gu8", "f8", "f8"]
CLASSES = ["bf", "gu8", "f8"]

_COMPILED = {}
_LAST = {}      # debug/profiling handle for test.py


def _gate_host(hs, gate_weight, bias):
    """numpy replica of reference._gate (verified bit-identical selection)."""
    T = hs.shape[0]
    logits = hs @ gate_weight.T                       # [T, E] fp32
    scores = 1.0 / (1.0 + np.exp(-logits))
    sfc = scores + bias[None, :]
    gs = sfc.reshape(T, G, E // G)
    gsort = np.sort(gs, axis=-1)
    group_scores = gsort[..., -1] + gsort[..., -2]
    group_idx = np.argsort(-group_scores, axis=-1, kind="stable")[:, :TOPK_GROUP]
    gmask = np.zeros((T, G), bool)
    gmask[np.arange(T)[:, None], group_idx] = True
    smask = np.repeat(gmask, E // G, axis=1)
    tmp = np.where(smask, sfc, 0.0)
    topk_idx = np.argsort(-tmp, axis=-1, kind="stable")[:, :TOP_K]
    topk_w = np.take_along_axis(scores, topk_idx, axis=1)
    topk_w = topk_w / (topk_w.sum(-1, keepdims=True) + 1e-20) * SCALE_FACTOR
    return topk_idx.astype(np.int32), topk_w.astype(np.float32)


def _blocks(C):
    """Split a batch of C tokens into NB equal blocks of width w (multiple
    of 8, <=512).  Returns (NB, w); capacity NB*w >= C."""
    if C == 0:
        return 0, 0
    NB = max(1, math.ceil(C / 512))
    w = math.ceil(C / (NB * 8)) * 8
    return NB, w


def _build(T, caps):
    """Build + compile the SPMD Bass program.

    caps[s][cls] = (NB, w) per slot s in (0, 1) and cls in CLASSES.
    """
    import concourse.mybir as mybir
    import concourse.tile as tile
    from concourse import bacc

    bf = mybir.dt.bfloat16
    f8 = mybir.dt.float8e4
    f32 = mybir.dt.float32
    AF = mybir.ActivationFunctionType
    DR = mybir.MatmulPerfMode.DoubleRow

    KH = H // P        # 16 contraction chunks over H
    MI = I // P        # 11 I chunks
    MH = H // P        # 16 output H chunks
    MS = S_SLICE // P  # 3
    NP_S = T // 1024   # shared token blocks

    def cw(s, cls):
        NB, w = caps[s][cls]
        return NB * w

    CB_tot = cw(0, "bf") + cw(1, "bf")
    C8_tot = sum(cw(s, c) for s in range(2) for c in ("gu8", "f8"))
    # column base of each (slot, cls) batch inside its dram tensor
    base_b = {0: 0, 1: cw(0, "bf")}
    base_8 = {}
    off = 0
    for s in range(2):
        for c in ("gu8", "f8"):
            base_8[(s, c)] = off
            off += cw(s, c)

    nc = bacc.Bacc("TRN2", target_bir_lowering=False, debug=False,
                   num_devices=N_CORES)
    xs = nc.dram_tensor("xs", [H, T], bf, kind="ExternalInput")
    xb = nc.dram_tensor("xb", [H, CB_tot], bf, kind="ExternalInput")
    x8 = nc.dram_tensor("x8", [H, C8_tot], f8, kind="ExternalInput")
    # weight panels pre-tiled on host to [tile, partition, ko*128+c]
    wg = nc.dram_tensor("wg", [EXP_PER_CORE * MI, P, KH * P], bf,
                        kind="ExternalInput")
    wu = nc.dram_tensor("wu", [EXP_PER_CORE * MI, P, KH * P], bf,
                        kind="ExternalInput")
    wd = nc.dram_tensor("wd", [EXP_PER_CORE * MH, P, MI * P], bf,
                        kind="ExternalInput")
    wg8 = nc.dram_tensor("wg8", [EXP_PER_CORE * MI, P, KH * P], f8,
                         kind="ExternalInput")
    wu8 = nc.dram_tensor("wu8", [EXP_PER_CORE * MI, P, KH * P], f8,
                         kind="ExternalInput")
    wd8 = nc.dram_tensor("wd8", [EXP_PER_CORE * MH, P, MI * P], f8,
                         kind="ExternalInput")
    sg = nc.dram_tensor("sg", [MS, P, KH * P], bf, kind="ExternalInput")
    su = nc.dram_tensor("su", [MS, P, KH * P], bf, kind="ExternalInput")
    sd = nc.dram_tensor("sd", [P, MS * H], bf, kind="ExternalInput")
    ye = nc.dram_tensor("ye", [H, CB_tot], bf, kind="ExternalOutput")
    ye8 = nc.dram_tensor("ye8", [H, C8_tot], bf, kind="ExternalOutput")
    ys = nc.dram_tensor("ys", [H, T], bf, kind="ExternalOutput")

    with tile.TileContext(nc) as tc:
        with (
            tc.tile_pool(name="xp", bufs=30) as xp,    # bf16 x tiles
            tc.tile_pool(name="x8p", bufs=1) as x8p,   # fp8 x tiles (1/slot)
            tc.tile_pool(name="swp", bufs=1) as swp,   # shared g/u (persistent)
            tc.tile_pool(name="wp", bufs=4) as wp,     # bf16 g/u weight cols
            tc.tile_pool(name="w8p", bufs=4) as w8p,   # fp8 g/u weight cols
            tc.tile_pool(name="wdp", bufs=3) as wdp,   # bf16 down cols
            tc.tile_pool(name="wd8p", bufs=2) as wd8p, # fp8 down cols
            tc.tile_pool(name="sdp", bufs=1) as sdp,   # shared down panel
            tc.tile_pool(name="itp", bufs=34) as itp,  # bf16 inter
            tc.tile_pool(name="it8p", bufs=10) as it8p,# fp8 inter
            tc.tile_pool(name="tmp", bufs=4) as tmp,   # silu/copy temps
            tc.tile_pool(name="otp", bufs=6) as otp,   # out staging
            tc.tile_pool(name="pg", bufs=2, space="PSUM") as pgp,
            tc.tile_pool(name="pu", bufs=2, space="PSUM") as pup,
            tc.tile_pool(name="py", bufs=4, space="PSUM") as pyp,
        ):
            # output DMAs: gpsimd, except the final down phase where we
            # alternate with the (by then idle) scalar queue to halve drain
            oqn = [0]
            tail_mode = [False]

            def odma(dst_ap, src_ap):
                if tail_mode[0]:
                    eng = (nc.gpsimd, nc.scalar)[oqn[0] & 1]
                    oqn[0] += 1
                else:
                    eng = nc.gpsimd
                eng.dma_start(dst_ap, src_ap)

            # ---------------- shared expert (column slice) ----------------
            # shared gate/up weight tiles are loaded ONCE (persistent pool),
            # in need-order interleaved with the x block loads
            swt = {}

            def load_sw(m):
                sgt = swp.tile([P, KH, P], bf, name=f"sgt{m}", tag=f"sg{m}")
                nc.sync.dma_start(
                    sgt[:], sg[m].rearrange("p (ko c) -> p ko c", c=P))
                sut = swp.tile([P, KH, P], bf, name=f"sut{m}", tag=f"su{m}")
                nc.gpsimd.dma_start(
                    sut[:], su[m].rearrange("p (ko c) -> p ko c", c=P))
                swt[m] = (sgt, sut)

            load_sw(0)
            first_engines = [nc.scalar, nc.sync, nc.gpsimd]
            blocks = [(0, 256), (256, 256), (512, 256), (768, 256)] + [
                (1024 * (b + 1), 1024) for b in range(NP_S - 1)]
            sdt = None
            for np_, (c0, bw) in enumerate(blocks):
                xst = []
                for k in range(KH):
                    t = xp.tile([P, 1024], bf, name=f"xs{np_}_{k}", tag="x")
                    eng = first_engines[k % 3] if np_ == 0 else nc.scalar
                    eng.dma_start(t[:, :bw], xs[k * P:(k + 1) * P, c0:c0 + bw])
                    xst.append(t)
                if np_ == 0:
                    load_sw(1)
                    load_sw(2)
                if sdt is None:
                    sdt = sdp.tile([P, MS, H], bf, name="sdt", tag="sdt")
                    nc.gpsimd.dma_start(
                        sdt[:], sd.ap().rearrange("p (ko c) -> p ko c", c=H))
                jw = min(512, bw)
                nj = bw // jw
                sint = {}
                for m in range(MS):
                    sgt, sut = swt[m]
                    for j in range(nj):
                        psg = pgp.tile([P, 512], f32, name=f"psgs{np_}_{m}{j}",
                                       tag="pg")
                        for k in range(KH):
                            nc.tensor.matmul(psg[:, :jw], sgt[:, k, :],
                                             xst[k][:, j * jw:(j + 1) * jw],
                                             start=(k == 0), stop=(k == KH - 1))
                        st = tmp.tile([P, 512], bf, name=f"sts{np_}_{m}{j}",
                                      tag="tmp")
                        nc.scalar.activation(st[:, :jw], psg[:, :jw], AF.Silu)
                        psu = pup.tile([P, 512], f32, name=f"psus{np_}_{m}{j}",
                                       tag="pu")
                        for k in range(KH):
                            nc.tensor.matmul(psu[:, :jw], sut[:, k, :],
                                             xst[k][:, j * jw:(j + 1) * jw],
                                             start=(k == 0), stop=(k == KH - 1))
                        it = itp.tile([P, 512], bf, name=f"si{np_}_{m}{j}",
                                      tag="it")
                        nc.vector.tensor_mul(it[:, :jw], st[:, :jw],
                                             psu[:, :jw])
                        sint[(m, j)] = it
                for M in range(MH):
                    ot = otp.tile([P, 1024], bf, name=f"ots{np_}_{M}", tag="ot")
                    for j in range(nj):
                        psy = pyp.tile([P, 512], f32, name=f"psys{np_}_{M}{j}",
                                       tag="py")
                        for K in range(MS):
                            nc.tensor.matmul(psy[:, :jw],
                                             sdt[:, K, M * P:(M + 1) * P],
                                             sint[(K, j)][:, :jw],
                                             start=(K == 0), stop=(K == MS - 1))
                        nc.vector.tensor_copy(ot[:, j * jw:(j + 1) * jw],
                                              psy[:, :jw])
                    (nc.gpsimd if M % 2 == 0 else nc.sync).dma_start(
                        ys[M * P:(M + 1) * P, c0:c0 + bw], ot[:, :bw])

            # ---------------- routed experts ----------------
            for s in range(2):
                NBb, wb = caps[s]["bf"]
                NBg, wg_ = caps[s]["gu8"]
                NBf, wf = caps[s]["f8"]
                Cb = NBb * wb
                C8s = NBg * wg_ + NBf * wf       # this slot's fp8 cols
                b8 = base_8[(s, "gu8")]           # gu8 then f8 contiguous

                # ---- x loads ----
                assert Cb <= 1024
                xbt = []
                for k in range(KH):
                    t = xp.tile([P, 1024], bf, name=f"xb{s}_{k}", tag="x")
                    nc.scalar.dma_start(
                        t[:, :Cb], xb[k * P:(k + 1) * P,
                                      base_b[s]:base_b[s] + Cb])
                    xbt.append(t)
                mx8 = max(
                    sum(caps[ss][cc][0] * caps[ss][cc][1]
                        for cc in ("gu8", "f8")) for ss in range(2))
                mx8 = (mx8 + 31) // 32 * 32
                x8t = x8p.tile([P, KH, mx8], f8, name=f"x8_{s}", tag="x8")
                for k in range(KH):
                    nc.scalar.dma_start(
                        x8t[:, k, :C8s], x8[k * P:(k + 1) * P, b8:b8 + C8s])

                # fp8 block list: (col0 within slot's x8 range, width, cls)
                fblk = [(i * wg_, wg_, "gu8") for i in range(NBg)] + \
                       [(NBg * wg_ + i * wf, wf, "f8") for i in range(NBf)]

                # ---- phase G: gate/up for all classes ----
                inter = {}    # bf16 inter: (cls, m, blk) -> tile
                inter8 = {}   # fp8 inter pair tiles: (blk, mp) -> tile
                for m in range(MI):
                    wgt = wp.tile([P, KH, P], bf, name=f"wgt{s}_{m}", tag="wp")
                    nc.sync.dma_start(wgt[:], wg[s * MI + m].rearrange("p (ko c) -> p ko c", c=P))
                    wut = wp.tile([P, KH, P], bf, name=f"wut{s}_{m}", tag="wp")
                    nc.sync.dma_start(wut[:], wu[s * MI + m].rearrange("p (ko c) -> p ko c", c=P))
                    wgt8 = w8p.tile([P, KH, P], f8, name=f"wgt8{s}_{m}", tag="w8")
                    nc.gpsimd.dma_start(wgt8[:], wg8[s * MI + m].rearrange("p (ko c) -> p ko c", c=P))
                    wut8 = w8p.tile([P, KH, P], f8, name=f"wut8{s}_{m}", tag="w8")
                    nc.gpsimd.dma_start(wut8[:], wu8[s * MI + m].rearrange("p (ko c) -> p ko c", c=P))

                    # bf16 blocks
                    for b in range(NBb):
                        c0 = b * wb
                        psg = pgp.tile([P, 512], f32, name=f"psg{s}_{m}_{b}",
                                       tag="pg")
                        for k in range(KH):
                            nc.tensor.matmul(psg[:, :wb], wgt[:, k, :],
                                             xbt[k][:, c0:c0 + wb],
                                             start=(k == 0), stop=(k == KH - 1))
                        st = tmp.tile([P, 512], bf, name=f"st{s}_{m}_{b}",
                                      tag="tmp")
                        nc.scalar.activation(st[:, :wb], psg[:, :wb], AF.Silu)
                        psu = pup.tile([P, 512], f32, name=f"psu{s}_{m}_{b}",
                                       tag="pu")
                        for k in range(KH):
                            nc.tensor.matmul(psu[:, :wb], wut[:, k, :],
                                             xbt[k][:, c0:c0 + wb],
                                             start=(k == 0), stop=(k == KH - 1))
                        it = itp.tile([P, 512], bf, name=f"it{s}_{m}_{b}",
                                      tag="it")
                        nc.vector.tensor_mul(it[:, :wb], st[:, :wb], psu[:, :wb])
                        inter[("bf", m, b)] = it

                    # fp8 blocks (gu8 + f8): DoubleRow gate/up
                    for bi, (c0, w, cls) in enumerate(fblk):
                        psg = pgp.tile([P, 512], f32, name=f"psg8{s}_{m}_{bi}",
                                       tag="pg")
                        for k in range(KH // 2):
                            nc.tensor.matmul(psg[:, :w],
                                             wgt8[:, 2 * k:2 * k + 2, :],
                                             x8t[:, 2 * k:2 * k + 2, c0:c0 + w],
                                             start=(k == 0),
                                             stop=(k == KH // 2 - 1),
                                             perf_mode=DR)
                        st = tmp.tile([P, 512], bf, name=f"st8{s}_{m}_{bi}",
                                      tag="tmp")
                        nc.scalar.activation(st[:, :w], psg[:, :w], AF.Silu,
                                             scale=1.0 / OS)
                        psu = pup.tile([P, 512], f32, name=f"psu8{s}_{m}_{bi}",
                                       tag="pu")
                        for k in range(KH // 2):
                            nc.tensor.matmul(psu[:, :w],
                                             wut8[:, 2 * k:2 * k + 2, :],
                                             x8t[:, 2 * k:2 * k + 2, c0:c0 + w],
                                             start=(k == 0),
                                             stop=(k == KH // 2 - 1),
                                             perf_mode=DR)
                        if cls == "gu8":
                            it = itp.tile([P, 512], bf,
                                          name=f"itg{s}_{m}_{bi}", tag="it")
                            nc.vector.tensor_mul(it[:, :w], st[:, :w],
                                                 psu[:, :w])
                            inter[("gu8", m, bi)] = it
                        else:
                            # fp8 inter at scale 16: ut = psu/1024 (=16*u)
                            ut = tmp.tile([P, 512], bf, name=f"ut{s}_{m}_{bi}",
                                          tag="tmp")
                            nc.scalar.activation(ut[:, :w], psu[:, :w],
                                                 AF.Copy, scale=1.0 / WS)
                            if m < 10:
                                mp, jj = m // 2, m % 2
                                key = (bi, mp)
                                if key not in inter8:
                                    inter8[key] = it8p.tile(
                                        [P, 2, 512], f8,
                                        name=f"it8{s}_{bi}_{mp}", tag="it8")
                                nc.vector.tensor_mul(
                                    inter8[key][:, jj, :w], st[:, :w],
                                    ut[:, :w])
                            else:
                                t = it8p.tile([P, 512], f8,
                                              name=f"it8L{s}_{bi}", tag="it8L",
                                              bufs=4)
                                nc.vector.tensor_mul(t[:, :w], st[:, :w],
                                                     ut[:, :w])
                                inter8[(bi, 5)] = t

                # ---- phase D: down for all classes ----
                if s == 1:
                    tail_mode[0] = True
                for M in range(MH):
                    wdt = wdp.tile([P, MI, P], bf, name=f"wdt{s}_{M}", tag="wdt")
                    nc.sync.dma_start(wdt[:], wd[s * MH + M].rearrange("p (ko c) -> p ko c", c=P))
                    wdt8 = wd8p.tile([P, MI, P], f8, name=f"wdt8{s}_{M}",
                                     tag="wd8")
                    nc.gpsimd.dma_start(wdt8[:], wd8[s * MH + M].rearrange("p (ko c) -> p ko c", c=P))

                    # bf16-down blocks: 'bf' and 'gu8' classes
                    for cls, NB_, w_, cbase in (
                            ("bf", NBb, wb, base_b[s]),
                            ("gu8", NBg, wg_, base_8[(s, "gu8")])):
                        dst = ye if cls == "bf" else ye8
                        for b in range(NB_):
                            c0 = cbase + b * w_
                            psy = pyp.tile([P, 512], f32,
                                           name=f"psy{s}_{M}_{cls}{b}",
                                           tag="py")
                            for K in range(MI):
                                nc.tensor.matmul(psy[:, :w_], wdt[:, K, :],
                                                 inter[(cls, K, b)][:, :w_],
                                                 start=(K == 0),
                                                 stop=(K == MI - 1))
                            ot = otp.tile([P, 512], bf,
                                          name=f"ot{s}_{M}_{cls}{b}", tag="ot")
                            nc.vector.tensor_copy(ot[:, :w_], psy[:, :w_])
                            odma(
                                dst[M * P:(M + 1) * P, c0:c0 + w_], ot[:, :w_])

                    # fp8-down blocks ('f8' class): 5 DoubleRow pairs + single
                    for bi, (c0_, w, cls) in enumerate(fblk):
                        if cls != "f8":
                            continue
                        c0 = base_8[(s, "gu8")] + c0_
                        psy = pyp.tile([P, 512], f32, name=f"psy8{s}_{M}_{bi}",
                                       tag="py")
                        for K in range(5):
                            nc.tensor.matmul(psy[:, :w],
                                             wdt8[:, 2 * K:2 * K + 2, :],
                                             inter8[(bi, K)][:, :, :w],
                                             start=(K == 0), stop=False,
                                             perf_mode=DR)
                        nc.tensor.matmul(psy[:, :w], wdt8[:, 10, :],
                                         inter8[(bi, 5)][:, :w],
                                         start=False, stop=True)
                        ot = otp.tile([P, 512], bf, name=f"ot8{s}_{M}_{bi}",
                                      tag="ot")
                        nc.vector.tensor_copy(ot[:, :w], psy[:, :w])
                        odma(
                            ye8[M * P:(M + 1) * P, c0:c0 + w], ot[:, :w])

    nc.compile()
    return nc


def _get_compiled(T, caps):
    key = (T, str(caps))
    if key not in _COMPILED:
        _COMPILED[key] = _build(T, caps)
    return _COMPILED[key]


def kernel(hidden_states, gate_weight, e_score_correction_bias,
           gate_proj, up_proj, down_proj,
           shared_gate_w, shared_up_w, shared_down_w):
    from concourse.bass_utils import run_bass_kernel_spmd

    hs = np.asarray(hidden_states, dtype=np.float32)
    B, S, Hh = hs.shape
    assert Hh == H
    hsf = np.ascontiguousarray(hs.reshape(-1, H))
    T = hsf.shape[0]
    gate_weight = np.asarray(gate_weight, np.float32)
    bias = np.asarray(e_score_correction_bias, np.float32)
    gate_proj = np.asarray(gate_proj, np.float32)
    up_proj = np.asarray(up_proj, np.float32)
    down_proj = np.asarray(down_proj, np.float32)
    shared_gate_w = np.asarray(shared_gate_w, np.float32)
    shared_up_w = np.asarray(shared_up_w, np.float32)
    shared_down_w = np.asarray(shared_down_w, np.float32)

    # ---- routing on host ----
    topk_idx, topk_w = _gate_host(hsf, gate_weight, bias)
    comb = np.zeros((T, E), np.float32)
    np.add.at(comb, (np.arange(T)[:, None], topk_idx), topk_w)
    rank = np.full((T, E), 99, np.int32)
    rank[np.arange(T)[:, None], topk_idx] = np.arange(TOP_K)[None, :]

    # per (expert, class) token lists
    idx_ec = {}
    cnt = np.zeros((E, len(CLASSES)), np.int64)
    for e in range(E):
        r = rank[:, e]
        valid = r < TOP_K
        for ci, cname in enumerate(CLASSES):
            m = valid & np.isin(r, [k for k in range(TOP_K)
                                    if RANK_CLASS[k] == cname])
            idx_ec[(e, cname)] = np.nonzero(m)[0]
            cnt[e, ci] = m.sum()

    # ---- slot assignment: brute-force the 8/8 partition that minimizes
    # weighted padded capacity (weights = PE cycles per token per class)
    CYC = {"bf": 528, "gu8": 352, "f8": 272}
    best, best_s0 = None, None
    all_e = frozenset(range(E))
    for s0 in combinations(range(E), N_CORES):
        s1 = tuple(sorted(all_e - set(s0)))
        tot = 0
        for ci, cname in enumerate(CLASSES):
            for sl in (s0, s1):
                NB, w = _blocks(max(cnt[list(sl), ci].max(), 8))
                tot += CYC[cname] * NB * w
        if best is None or tot < best:
            best, best_s0 = tot, s0
    s0 = list(best_s0)
    s1 = sorted(all_e - set(s0))
    assign = np.stack([np.array(s0), np.array(s1)], axis=1)  # [core, slot]

    caps = []
    for sl in (s0, s1):
        d = {}
        for ci, cname in enumerate(CLASSES):
            d[cname] = _blocks(max(cnt[list(sl), ci].max(), 8))
        caps.append(d)

    def cwid(s, c):
        NB, w = caps[s][c]
        return NB * w

    CB_tot = cwid(0, "bf") + cwid(1, "bf")
    C8_tot = sum(cwid(s, c) for s in range(2) for c in ("gu8", "f8"))
    base_b = {0: 0, 1: cwid(0, "bf")}
    base_8 = {}
    off = 0
    for s in range(2):
        for c in ("gu8", "f8"):
            base_8[(s, c)] = off
            off += cwid(s, c)

    # ---- host-side dispatch (shard + transpose + casts) ----
    xsT = np.ascontiguousarray(hsf.T).astype(BF16)          # [H, T] bf16
    xsT8 = np.asarray(
        np.ascontiguousarray(hsf.T) * XS, F8)               # [H, T] fp8*16

    MI_, MH_, MS_, KH_ = I // P, H // P, S_SLICE // P, H // P

    def tile_gu(wmat):  # [I, H] -> [MI, P, KH*P]
        return np.ascontiguousarray(
            wmat.reshape(MI_, P, KH_, P).transpose(0, 3, 2, 1)
        ).reshape(MI_, P, KH_ * P)

    def tile_dn(wmat):  # [H, I] -> [MH, P, MI*P]
        return np.ascontiguousarray(
            wmat.reshape(MH_, P, MI_, P).transpose(0, 3, 2, 1)
        ).reshape(MH_, P, MI_ * P)

    in_maps = []
    for c in range(N_CORES):
        e0, e1 = assign[c]
        xb_c = np.zeros((H, CB_tot), BF16)
        x8_c = np.zeros((H, C8_tot), F8)
        for sl, e in enumerate((e0, e1)):
            ib = idx_ec[(e, "bf")]
            xb_c[:, base_b[sl]:base_b[sl] + len(ib)] = xsT[:, ib]
            for cname in ("gu8", "f8"):
                ix = idx_ec[(e, cname)]
                b0 = base_8[(sl, cname)]
                x8_c[:, b0:b0 + len(ix)] = xsT8[:, ix]
        wg_c = np.concatenate([tile_gu(gate_proj[e]).astype(BF16)
                               for e in (e0, e1)])
        wu_c = np.concatenate([tile_gu(up_proj[e]).astype(BF16)
                               for e in (e0, e1)])
        wd_c = np.concatenate([tile_dn(down_proj[e]).astype(BF16)
                               for e in (e0, e1)])
        wg8_c = np.concatenate([np.asarray(tile_gu(gate_proj[e]) * WS, F8)
                                for e in (e0, e1)])
        wu8_c = np.concatenate([np.asarray(tile_gu(up_proj[e]) * WS, F8)
                                for e in (e0, e1)])
        wd8_c = np.concatenate([np.asarray(tile_dn(down_proj[e]) * WS, F8)
                                for e in (e0, e1)])
        r0, r1 = c * S_SLICE_RAW, (c + 1) * S_SLICE_RAW
        sgp = np.zeros((S_SLICE, H), np.float32)
        sup = np.zeros((S_SLICE, H), np.float32)
        sdpn = np.zeros((S_SLICE, H), np.float32)
        sgp[:S_SLICE_RAW] = shared_gate_w[r0:r1, :]
        sup[:S_SLICE_RAW] = shared_up_w[r0:r1, :]
        sdpn[:S_SLICE_RAW] = shared_down_w[:, r0:r1].T
        sg_c = np.ascontiguousarray(
            sgp.reshape(MS_, P, KH_, P).transpose(0, 3, 2, 1)
        ).reshape(MS_, P, KH_ * P).astype(BF16)
        su_c = np.ascontiguousarray(
            sup.reshape(MS_, P, KH_, P).transpose(0, 3, 2, 1)
        ).reshape(MS_, P, KH_ * P).astype(BF16)
        sd_c = np.ascontiguousarray(
            sdpn.reshape(MS_, P, H).transpose(1, 0, 2)
        ).reshape(P, MS_ * H).astype(BF16)
        in_maps.append({
            "xs": xsT, "xb": xb_c, "x8": x8_c,
            "wg": wg_c, "wu": wu_c, "wd": wd_c,
            "wg8": wg8_c, "wu8": wu8_c, "wd8": wd8_c,
            "sg": sg_c, "su": su_c, "sd": sd_c,
        })

    nc = _get_compiled(T, caps)
    results = run_bass_kernel_spmd(nc, in_maps, core_ids=list(range(N_CORES)))

    _LAST.clear()
    _LAST.update(nc=nc, in_maps=in_maps, results=results, caps=caps)

    # ---- host-side combine ----
    outT = np.zeros((H, T), np.float32)
    for c in range(N_CORES):
        outT += results.results[c]["ys"].astype(np.float32)
    for c in range(N_CORES):
        yev = results.results[c]["ye"].astype(np.float32)
        ye8v = results.results[c]["ye8"].astype(np.float32)
        for sl in range(EXP_PER_CORE):
            e = assign[c, sl]
            ib = idx_ec[(e, "bf")]
            if len(ib):
                we = comb[ib, e]
                b0 = base_b[sl]
                outT[:, ib] += yev[:, b0:b0 + len(ib)] * we[None, :]
            for cname in ("gu8", "f8"):
                ix = idx_ec[(e, cname)]
                if len(ix) == 0:
                    continue
                we = comb[ix, e] / OS
                b0 = base_8[(sl, cname)]
                outT[:, ix] += ye8v[:, b0:b0 + len(ix)] * we[None, :]

    return np.ascontiguousarray(outT.T).reshape(B, S, H).astype(np.float32)


# revision 17
# speedup vs baseline: 1.1899x; 1.1899x over previous
"""MoE routing kernel for Trainium2 (8 NeuronCores, SPMD expert-parallel).

Contract: kernel(**full_inputs) -> full output [B, S, H] float32.

Strategy
--------
- Host: gate + group-topk routing in numpy (bit-identical selection to the
  jax reference), then dispatch: gather each expert's tokens into padded,
  transposed buffers (the "all-to-all by topk_idx" of the sharding hint).
- Device (SPMD over 8 cores): core c holds 2 experts and a 1/8 column
  slice of the shared expert.  Mixed precision by routing rank: tokens for
  which this expert is one of their top-3 choices run the full SwiGLU in
  bf16; rank-3 tokens run gate/up in fp8-e4m3 (DoubleRow, 2x PE rate) with
  a bf16 down; rank-4/5 tokens run everything in fp8.  The fp8 error is
  attenuated by the (smaller) routing weights of the low-rank experts, so
  the end-to-end error stays ~1.8e-2 < 2e-2.
- Host: scale per-expert outputs by routing weights (and the fp8 output
  scale), scatter-add over token indices, add the 8 shared partials.

All matmuls accumulate in fp32 PSUM.  fp8 scales: x*16, w*1024; silu gets
scale=1/16384 on its input; the residual 16384 output factor folds into
the host-side combine weights.
"""

import math
from itertools import combinations

import numpy as np
import ml_dtypes

H = 2048          # hidden size
I = 1408          # intermediate per routed expert
E = 16            # routed experts
G = 4             # groups
TOPK_GROUP = 2
TOP_K = 6
N_SHARED = 2
SCALE_FACTOR = 2.5
SI = I * N_SHARED  # 2816 shared intermediate
N_CORES = 8
EXP_PER_CORE = E // N_CORES  # 2
S_SLICE_RAW = SI // N_CORES  # 352
S_SLICE = 384                # padded to 3*128
P = 128
BF16 = ml_dtypes.bfloat16
F8 = ml_dtypes.float8_e4m3   # TRN FP8_EXP4 (max 240)

XS = 16.0      # fp8 activation scale
WS = 1024.0    # fp8 weight scale
OS = XS * WS   # 16384: scale of fp8-path outputs (divided out on host)

# rank -> class: 'bf' (all bf16), 'gu8' (fp8 gate/up, bf16 down),
# 'f8' (all fp8).  Tune for error budget: sim says this lands ~1.8e-2.
RANK_CLASS = ["bf", "bf", "bf", "gu8", "f8", "f8"]
CLASSES = ["bf", "gu8", "f8"]

_COMPILED = {}
_LAST = {}      # debug/profiling handle for test.py


def _gate_host(hs, gate_weight, bias):
    """numpy replica of reference._gate (verified bit-identical selection)."""
    T = hs.shape[0]
    logits = hs @ gate_weight.T                       # [T, E] fp32
    scores = 1.0 / (1.0 + np.exp(-logits))
    sfc = scores + bias[None, :]
    gs = sfc.reshape(T, G, E // G)
    gsort = np.sort(gs, axis=-1)
    group_scores = gsort[..., -1] + gsort[..., -2]
    group_idx = np.argsort(-group_scores, axis=-1, kind="stable")[:, :TOPK_GROUP]
    gmask = np.zeros((T, G), bool)
    gmask[np.arange(T)[:, None], group_idx] = True
    smask = np.repeat(gmask, E // G, axis=1)
    tmp = np.where(smask, sfc, 0.0)
    topk_idx = np.argsort(-tmp, axis=-1, kind="stable")[:, :TOP_K]
    topk_w = np.take_along_axis(scores, topk_idx, axis=1)
    topk_w = topk_w / (topk_w.sum(-1, keepdims=True) + 1e-20) * SCALE_FACTOR
    return topk_idx.astype(np.int32), topk_w.astype(np.float32)


def _blocks(C):
    """Split a batch of C tokens into NB equal blocks of width w (multiple
    of 8, <=512).  Returns (NB, w); capacity NB*w >= C."""
    if C == 0:
        return 0, 0
    NB = max(1, math.ceil(C / 512))
    w = math.ceil(C / (NB * 8)) * 8
    return NB, w


def _build(T, caps):
    """Build + compile the SPMD Bass program.

    caps[s][cls] = (NB, w) per slot s in (0, 1) and cls in CLASSES.
    """
    import concourse.mybir as mybir
    import concourse.tile as tile
    from concourse import bacc

    bf = mybir.dt.bfloat16
    f8 = mybir.dt.float8e4
    f32 = mybir.dt.float32
    AF = mybir.ActivationFunctionType
    DR = mybir.MatmulPerfMode.DoubleRow

    KH = H // P        # 16 contraction chunks over H
    MI = I // P        # 11 I chunks
    MH = H // P        # 16 output H chunks
    MS = S_SLICE // P  # 3
    NP_S = T // 1024   # shared token blocks

    def cw(s, cls):
        NB, w = caps[s][cls]
        return NB * w

    CB_tot = cw(0, "bf") + cw(1, "bf")
    C8_tot = sum(cw(s, c) for s in range(2) for c in ("gu8", "f8"))
    # column base of each (slot, cls) batch inside its dram tensor
    base_b = {0: 0, 1: cw(0, "bf")}
    base_8 = {}
    off = 0
    for s in range(2):
        for c in ("gu8", "f8"):
            base_8[(s, c)] = off
            off += cw(s, c)

    nc = bacc.Bacc("TRN2", target_bir_lowering=False, debug=False,
                   num_devices=N_CORES)
    xs = nc.dram_tensor("xs", [H, T], bf, kind="ExternalInput")
    xb = nc.dram_tensor("xb", [H, CB_tot], bf, kind="ExternalInput")
    x8 = nc.dram_tensor("x8", [H, C8_tot], f8, kind="ExternalInput")
    # weight panels pre-tiled on host to [tile, partition, ko*128+c]
    wg = nc.dram_tensor("wg", [EXP_PER_CORE * MI, P, KH * P], bf,
                        kind="ExternalInput")
    wu = nc.dram_tensor("wu", [EXP_PER_CORE * MI, P, KH * P], bf,
                        kind="ExternalInput")
    wd = nc.dram_tensor("wd", [EXP_PER_CORE * MH, P, MI * P], bf,
                        kind="ExternalInput")
    wg8 = nc.dram_tensor("wg8", [EXP_PER_CORE * MI, P, KH * P], f8,
                         kind="ExternalInput")
    wu8 = nc.dram_tensor("wu8", [EXP_PER_CORE * MI, P, KH * P], f8,
                         kind="ExternalInput")
    wd8 = nc.dram_tensor("wd8", [EXP_PER_CORE * MH, P, MI * P], f8,
                         kind="ExternalInput")
    sg = nc.dram_tensor("sg", [MS, P, KH * P], bf, kind="ExternalInput")
    su = nc.dram_tensor("su", [MS, P, KH * P], bf, kind="ExternalInput")
    sd = nc.dram_tensor("sd", [P, MS * H], bf, kind="ExternalInput")
    ye = nc.dram_tensor("ye", [H, CB_tot], bf, kind="ExternalOutput")
    ye8 = nc.dram_tensor("ye8", [H, C8_tot], bf, kind="ExternalOutput")
    ys = nc.dram_tensor("ys", [H, T], bf, kind="ExternalOutput")

    with tile.TileContext(nc) as tc:
        with (
            tc.tile_pool(name="xp", bufs=30) as xp,    # bf16 x tiles
            tc.tile_pool(name="x8p", bufs=1) as x8p,   # fp8 x tiles (1/slot)
            tc.tile_pool(name="swp", bufs=1) as swp,   # shared g/u (persistent)
            tc.tile_pool(name="wp", bufs=4) as wp,     # bf16 g/u weight cols
            tc.tile_pool(name="w8p", bufs=4) as w8p,   # fp8 g/u weight cols
            tc.tile_pool(name="wdp", bufs=3) as wdp,   # bf16 down cols
            tc.tile_pool(name="wd8p", bufs=2) as wd8p, # fp8 down cols
            tc.tile_pool(name="sdp", bufs=1) as sdp,   # shared down panel
            tc.tile_pool(name="itp", bufs=34) as itp,  # bf16 inter
            tc.tile_pool(name="it8p", bufs=10) as it8p,# fp8 inter
            tc.tile_pool(name="tmp", bufs=4) as tmp,   # silu/copy temps
            tc.tile_pool(name="otp", bufs=6) as otp,   # out staging
            tc.tile_pool(name="pg", bufs=2, space="PSUM") as pgp,
            tc.tile_pool(name="pu", bufs=2, space="PSUM") as pup,
            tc.tile_pool(name="py", bufs=4, space="PSUM") as pyp,
        ):
            # output DMAs: gpsimd, except the final down phase where we
            # alternate with the (by then idle) scalar queue to halve drain
            oqn = [0]
            tail_mode = [False]

            def odma(dst_ap, src_ap):
                if tail_mode[0]:
                    eng = (nc.gpsimd, nc.scalar)[oqn[0] & 1]
                    oqn[0] += 1
                else:
                    eng = nc.gpsimd
                eng.dma_start(dst_ap, src_ap)

            # ---------------- shared expert (column slice) ----------------
            # shared gate/up weight tiles are loaded ONCE (persistent pool),
            # in need-order interleaved with the x block loads
            swt = {}

            def load_sw(m):
                sgt = swp.tile([P, KH, P], bf, name=f"sgt{m}", tag=f"sg{m}")
                nc.sync.dma_start(
                    sgt[:], sg[m].rearrange("p (ko c) -> p ko c", c=P))
                sut = swp.tile([P, KH, P], bf, name=f"sut{m}", tag=f"su{m}")
                nc.gpsimd.dma_start(
                    sut[:], su[m].rearrange("p (ko c) -> p ko c", c=P))
                swt[m] = (sgt, sut)

            load_sw(0)
            first_engines = [nc.scalar, nc.sync, nc.gpsimd]
            blocks = [(0, 512), (512, 512)] + [
                (1024 * (b + 1), 1024) for b in range(NP_S - 1)]
            sdt = None
            for np_, (c0, bw) in enumerate(blocks):
                xst = []
                for k in range(KH):
                    t = xp.tile([P, 1024], bf, name=f"xs{np_}_{k}", tag="x")
                    eng = first_engines[k % 3] if np_ == 0 else nc.scalar
                    eng.dma_start(t[:, :bw], xs[k * P:(k + 1) * P, c0:c0 + bw])
                    xst.append(t)
                if np_ == 0:
                    load_sw(1)
                    load_sw(2)
                if sdt is None:
                    sdt = sdp.tile([P, MS, H], bf, name="sdt", tag="sdt")
                    nc.gpsimd.dma_start(
                        sdt[:], sd.ap().rearrange("p (ko c) -> p ko c", c=H))
                jw = min(512, bw)
                nj = bw // jw
                sint = {}
                for m in range(MS):
                    sgt, sut = swt[m]
                    for j in range(nj):
                        psg = pgp.tile([P, 512], f32, name=f"psgs{np_}_{m}{j}",
                                       tag="pg")
                        for k in range(KH):
                            nc.tensor.matmul(psg[:, :jw], sgt[:, k, :],
                                             xst[k][:, j * jw:(j + 1) * jw],
                                             start=(k == 0), stop=(k == KH - 1))
                        st = tmp.tile([P, 512], bf, name=f"sts{np_}_{m}{j}",
                                      tag="tmp")
                        nc.scalar.activation(st[:, :jw], psg[:, :jw], AF.Silu)
                        psu = pup.tile([P, 512], f32, name=f"psus{np_}_{m}{j}",
                                       tag="pu")
                        for k in range(KH):
                            nc.tensor.matmul(psu[:, :jw], sut[:, k, :],
                                             xst[k][:, j * jw:(j + 1) * jw],
                                             start=(k == 0), stop=(k == KH - 1))
                        it = itp.tile([P, 512], bf, name=f"si{np_}_{m}{j}",
                                      tag="it")
                        nc.vector.tensor_mul(it[:, :jw], st[:, :jw],
                                             psu[:, :jw])
                        sint[(m, j)] = it
                for M in range(MH):
                    ot = otp.tile([P, 1024], bf, name=f"ots{np_}_{M}", tag="ot")
                    for j in range(nj):
                        psy = pyp.tile([P, 512], f32, name=f"psys{np_}_{M}{j}",
                                       tag="py")
                        for K in range(MS):
                            nc.tensor.matmul(psy[:, :jw],
                                             sdt[:, K, M * P:(M + 1) * P],
                                             sint[(K, j)][:, :jw],
                                             start=(K == 0), stop=(K == MS - 1))
                        nc.vector.tensor_copy(ot[:, j * jw:(j + 1) * jw],
                                              psy[:, :jw])
                    (nc.gpsimd if M % 2 == 0 else nc.sync).dma_start(
                        ys[M * P:(M + 1) * P, c0:c0 + bw], ot[:, :bw])

            # ---------------- routed experts ----------------
            for s in range(2):
                NBb, wb = caps[s]["bf"]
                NBg, wg_ = caps[s]["gu8"]
                NBf, wf = caps[s]["f8"]
                Cb = NBb * wb
                C8s = NBg * wg_ + NBf * wf       # this slot's fp8 cols
                b8 = base_8[(s, "gu8")]           # gu8 then f8 contiguous

                # ---- x loads ----
                assert Cb <= 1024
                xbt = []
                for k in range(KH):
                    t = xp.tile([P, 1024], bf, name=f"xb{s}_{k}", tag="x")
                    nc.scalar.dma_start(
                        t[:, :Cb], xb[k * P:(k + 1) * P,
                                      base_b[s]:base_b[s] + Cb])
                    xbt.append(t)
                mx8 = max(
                    sum(caps[ss][cc][0] * caps[ss][cc][1]
                        for cc in ("gu8", "f8")) for ss in range(2))
                mx8 = (mx8 + 31) // 32 * 32
                x8t = x8p.tile([P, KH, mx8], f8, name=f"x8_{s}", tag="x8")
                for k in range(KH):
                    nc.scalar.dma_start(
                        x8t[:, k, :C8s], x8[k * P:(k + 1) * P, b8:b8 + C8s])

                # fp8 block list: (col0 within slot's x8 range, width, cls)
                fblk = [(i * wg_, wg_, "gu8") for i in range(NBg)] + \
                       [(NBg * wg_ + i * wf, wf, "f8") for i in range(NBf)]

                # ---- phase G: gate/up for all classes ----
                inter = {}    # bf16 inter: (cls, m, blk) -> tile
                inter8 = {}   # fp8 inter pair tiles: (blk, mp) -> tile
                for m in range(MI):
                    wgt = wp.tile([P, KH, P], bf, name=f"wgt{s}_{m}", tag="wp")
                    nc.sync.dma_start(wgt[:], wg[s * MI + m].rearrange("p (ko c) -> p ko c", c=P))
                    wut = wp.tile([P, KH, P], bf, name=f"wut{s}_{m}", tag="wp")
                    nc.sync.dma_start(wut[:], wu[s * MI + m].rearrange("p (ko c) -> p ko c", c=P))
                    wgt8 = w8p.tile([P, KH, P], f8, name=f"wgt8{s}_{m}", tag="w8")
                    nc.gpsimd.dma_start(wgt8[:], wg8[s * MI + m].rearrange("p (ko c) -> p ko c", c=P))
                    wut8 = w8p.tile([P, KH, P], f8, name=f"wut8{s}_{m}", tag="w8")
                    nc.gpsimd.dma_start(wut8[:], wu8[s * MI + m].rearrange("p (ko c) -> p ko c", c=P))

                    # bf16 blocks
                    for b in range(NBb):
                        c0 = b * wb
                        psg = pgp.tile([P, 512], f32, name=f"psg{s}_{m}_{b}",
                                       tag="pg")
                        for k in range(KH):
                            nc.tensor.matmul(psg[:, :wb], wgt[:, k, :],
                                             xbt[k][:, c0:c0 + wb],
                                             start=(k == 0), stop=(k == KH - 1))
                        st = tmp.tile([P, 512], bf, name=f"st{s}_{m}_{b}",
                                      tag="tmp")
                        nc.scalar.activation(st[:, :wb], psg[:, :wb], AF.Silu)
                        psu = pup.tile([P, 512], f32, name=f"psu{s}_{m}_{b}",
                                       tag="pu")
                        for k in range(KH):
                            nc.tensor.matmul(psu[:, :wb], wut[:, k, :],
                                             xbt[k][:, c0:c0 + wb],
                                             start=(k == 0), stop=(k == KH - 1))
                        it = itp.tile([P, 512], bf, name=f"it{s}_{m}_{b}",
                                      tag="it")
                        nc.vector.tensor_mul(it[:, :wb], st[:, :wb], psu[:, :wb])
                        inter[("bf", m, b)] = it

                    # fp8 blocks (gu8 + f8): DoubleRow gate/up
                    for bi, (c0, w, cls) in enumerate(fblk):
                        psg = pgp.tile([P, 512], f32, name=f"psg8{s}_{m}_{bi}",
                                       tag="pg")
                        for k in range(KH // 2):
                            nc.tensor.matmul(psg[:, :w],
                                             wgt8[:, 2 * k:2 * k + 2, :],
                                             x8t[:, 2 * k:2 * k + 2, c0:c0 + w],
                                             start=(k == 0),
                                             stop=(k == KH // 2 - 1),
                                             perf_mode=DR)
                        st = tmp.tile([P, 512], bf, name=f"st8{s}_{m}_{bi}",
                                      tag="tmp")
                        nc.scalar.activation(st[:, :w], psg[:, :w], AF.Silu,
                                             scale=1.0 / OS)
                        psu = pup.tile([P, 512], f32, name=f"psu8{s}_{m}_{bi}",
                                       tag="pu")
                        for k in range(KH // 2):
                            nc.tensor.matmul(psu[:, :w],
                                             wut8[:, 2 * k:2 * k + 2, :],
                                             x8t[:, 2 * k:2 * k + 2, c0:c0 + w],
                                             start=(k == 0),
                                             stop=(k == KH // 2 - 1),
                                             perf_mode=DR)
                        if cls == "gu8":
                            it = itp.tile([P, 512], bf,
                                          name=f"itg{s}_{m}_{bi}", tag="it")
                            nc.vector.tensor_mul(it[:, :w], st[:, :w],
                                                 psu[:, :w])
                            inter[("gu8", m, bi)] = it
                        else:
                            # fp8 inter at scale 16: ut = psu/1024 (=16*u)
                            ut = tmp.tile([P, 512], bf, name=f"ut{s}_{m}_{bi}",
                                          tag="tmp")
                            nc.scalar.activation(ut[:, :w], psu[:, :w],
                                                 AF.Copy, scale=1.0 / WS)
                            if m < 10:
                                mp, jj = m // 2, m % 2
                                key = (bi, mp)
                                if key not in inter8:
                                    inter8[key] = it8p.tile(
                                        [P, 2, 512], f8,
                                        name=f"it8{s}_{bi}_{mp}", tag="it8")
                                nc.vector.tensor_mul(
                                    inter8[key][:, jj, :w], st[:, :w],
                                    ut[:, :w])
                            else:
                                t = it8p.tile([P, 512], f8,
                                              name=f"it8L{s}_{bi}", tag="it8L",
                                              bufs=4)
                                nc.vector.tensor_mul(t[:, :w], st[:, :w],
                                                     ut[:, :w])
                                inter8[(bi, 5)] = t

                # ---- phase D: down for all classes ----
                if s == 1:
                    tail_mode[0] = True
                for M in range(MH):
                    wdt = wdp.tile([P, MI, P], bf, name=f"wdt{s}_{M}", tag="wdt")
                    nc.sync.dma_start(wdt[:], wd[s * MH + M].rearrange("p (ko c) -> p ko c", c=P))
                    wdt8 = wd8p.tile([P, MI, P], f8, name=f"wdt8{s}_{M}",
                                     tag="wd8")
                    nc.gpsimd.dma_start(wdt8[:], wd8[s * MH + M].rearrange("p (ko c) -> p ko c", c=P))

                    # bf16-down blocks: 'bf' and 'gu8' classes
                    for cls, NB_, w_, cbase in (
                            ("bf", NBb, wb, base_b[s]),
                            ("gu8", NBg, wg_, base_8[(s, "gu8")])):
                        dst = ye if cls == "bf" else ye8
                        for b in range(NB_):
                            c0 = cbase + b * w_
                            psy = pyp.tile([P, 512], f32,
                                           name=f"psy{s}_{M}_{cls}{b}",
                                           tag="py")
                            for K in range(MI):
                                nc.tensor.matmul(psy[:, :w_], wdt[:, K, :],
                                                 inter[(cls, K, b)][:, :w_],
                                                 start=(K == 0),
                                                 stop=(K == MI - 1))
                            ot = otp.tile([P, 512], bf,
                                          name=f"ot{s}_{M}_{cls}{b}", tag="ot")
                            nc.vector.tensor_copy(ot[:, :w_], psy[:, :w_])
                            odma(
                                dst[M * P:(M + 1) * P, c0:c0 + w_], ot[:, :w_])

                    # fp8-down blocks ('f8' class): 5 DoubleRow pairs + single
                    for bi, (c0_, w, cls) in enumerate(fblk):
                        if cls != "f8":
                            continue
                        c0 = base_8[(s, "gu8")] + c0_
                        psy = pyp.tile([P, 512], f32, name=f"psy8{s}_{M}_{bi}",
                                       tag="py")
                        for K in range(5):
                            nc.tensor.matmul(psy[:, :w],
                                             wdt8[:, 2 * K:2 * K + 2, :],
                                             inter8[(bi, K)][:, :, :w],
                                             start=(K == 0), stop=False,
                                             perf_mode=DR)
                        nc.tensor.matmul(psy[:, :w], wdt8[:, 10, :],
                                         inter8[(bi, 5)][:, :w],
                                         start=False, stop=True)
                        ot = otp.tile([P, 512], bf, name=f"ot8{s}_{M}_{bi}",
                                      tag="ot")
                        nc.vector.tensor_copy(ot[:, :w], psy[:, :w])
                        odma(
                            ye8[M * P:(M + 1) * P, c0:c0 + w], ot[:, :w])

    nc.compile()
    return nc


def _get_compiled(T, caps):
    key = (T, str(caps))
    if key not in _COMPILED:
        _COMPILED[key] = _build(T, caps)
    return _COMPILED[key]


def kernel(hidden_states, gate_weight, e_score_correction_bias,
           gate_proj, up_proj, down_proj,
           shared_gate_w, shared_up_w, shared_down_w):
    from concourse.bass_utils import run_bass_kernel_spmd

    hs = np.asarray(hidden_states, dtype=np.float32)
    B, S, Hh = hs.shape
    assert Hh == H
    hsf = np.ascontiguousarray(hs.reshape(-1, H))
    T = hsf.shape[0]
    gate_weight = np.asarray(gate_weight, np.float32)
    bias = np.asarray(e_score_correction_bias, np.float32)
    gate_proj = np.asarray(gate_proj, np.float32)
    up_proj = np.asarray(up_proj, np.float32)
    down_proj = np.asarray(down_proj, np.float32)
    shared_gate_w = np.asarray(shared_gate_w, np.float32)
    shared_up_w = np.asarray(shared_up_w, np.float32)
    shared_down_w = np.asarray(shared_down_w, np.float32)

    # ---- routing on host ----
    topk_idx, topk_w = _gate_host(hsf, gate_weight, bias)
    comb = np.zeros((T, E), np.float32)
    np.add.at(comb, (np.arange(T)[:, None], topk_idx), topk_w)
    rank = np.full((T, E), 99, np.int32)
    rank[np.arange(T)[:, None], topk_idx] = np.arange(TOP_K)[None, :]

    # per (expert, class) token lists
    idx_ec = {}
    cnt = np.zeros((E, len(CLASSES)), np.int64)
    for e in range(E):
        r = rank[:, e]
        valid = r < TOP_K
        for ci, cname in enumerate(CLASSES):
            m = valid & np.isin(r, [k for k in range(TOP_K)
                                    if RANK_CLASS[k] == cname])
            idx_ec[(e, cname)] = np.nonzero(m)[0]
            cnt[e, ci] = m.sum()

    # ---- slot assignment: brute-force the 8/8 partition that minimizes
    # weighted padded capacity (weights = PE cycles per token per class)
    CYC = {"bf": 528, "gu8": 352, "f8": 272}
    best, best_s0 = None, None
    all_e = frozenset(range(E))
    for s0 in combinations(range(E), N_CORES):
        s1 = tuple(sorted(all_e - set(s0)))
        tot = 0
        for ci, cname in enumerate(CLASSES):
            for sl in (s0, s1):
                NB, w = _blocks(max(cnt[list(sl), ci].max(), 8))
                tot += CYC[cname] * NB * w
        if best is None or tot < best:
            best, best_s0 = tot, s0
    s0 = list(best_s0)
    s1 = sorted(all_e - set(s0))
    assign = np.stack([np.array(s0), np.array(s1)], axis=1)  # [core, slot]

    caps = []
    for sl in (s0, s1):
        d = {}
        for ci, cname in enumerate(CLASSES):
            d[cname] = _blocks(max(cnt[list(sl), ci].max(), 8))
        caps.append(d)

    def cwid(s, c):
        NB, w = caps[s][c]
        return NB * w

    CB_tot = cwid(0, "bf") + cwid(1, "bf")
    C8_tot = sum(cwid(s, c) for s in range(2) for c in ("gu8", "f8"))
    base_b = {0: 0, 1: cwid(0, "bf")}
    base_8 = {}
    off = 0
    for s in range(2):
        for c in ("gu8", "f8"):
            base_8[(s, c)] = off
            off += cwid(s, c)

    # ---- host-side dispatch (shard + transpose + casts) ----
    xsT = np.ascontiguousarray(hsf.T).astype(BF16)          # [H, T] bf16
    xsT8 = np.asarray(
        np.ascontiguousarray(hsf.T) * XS, F8)               # [H, T] fp8*16

    MI_, MH_, MS_, KH_ = I // P, H // P, S_SLICE // P, H // P

    def tile_gu(wmat):  # [I, H] -> [MI, P, KH*P]
        return np.ascontiguousarray(
            wmat.reshape(MI_, P, KH_, P).transpose(0, 3, 2, 1)
        ).reshape(MI_, P, KH_ * P)

    def tile_dn(wmat):  # [H, I] -> [MH, P, MI*P]
        return np.ascontiguousarray(
            wmat.reshape(MH_, P, MI_, P).transpose(0, 3, 2, 1)
        ).reshape(MH_, P, MI_ * P)

    in_maps = []
    for c in range(N_CORES):
        e0, e1 = assign[c]
        xb_c = np.zeros((H, CB_tot), BF16)
        x8_c = np.zeros((H, C8_tot), F8)
        for sl, e in enumerate((e0, e1)):
            ib = idx_ec[(e, "bf")]
            xb_c[:, base_b[sl]:base_b[sl] + len(ib)] = xsT[:, ib]
            for cname in ("gu8", "f8"):
                ix = idx_ec[(e, cname)]
                b0 = base_8[(sl, cname)]
                x8_c[:, b0:b0 + len(ix)] = xsT8[:, ix]
        wg_c = np.concatenate([tile_gu(gate_proj[e]).astype(BF16)
                               for e in (e0, e1)])
        wu_c = np.concatenate([tile_gu(up_proj[e]).astype(BF16)
                               for e in (e0, e1)])
        wd_c = np.concatenate([tile_dn(down_proj[e]).astype(BF16)
                               for e in (e0, e1)])
        wg8_c = np.concatenate([np.asarray(tile_gu(gate_proj[e]) * WS, F8)
                                for e in (e0, e1)])
        wu8_c = np.concatenate([np.asarray(tile_gu(up_proj[e]) * WS, F8)
                                for e in (e0, e1)])
        wd8_c = np.concatenate([np.asarray(tile_dn(down_proj[e]) * WS, F8)
                                for e in (e0, e1)])
        r0, r1 = c * S_SLICE_RAW, (c + 1) * S_SLICE_RAW
        sgp = np.zeros((S_SLICE, H), np.float32)
        sup = np.zeros((S_SLICE, H), np.float32)
        sdpn = np.zeros((S_SLICE, H), np.float32)
        sgp[:S_SLICE_RAW] = shared_gate_w[r0:r1, :]
        sup[:S_SLICE_RAW] = shared_up_w[r0:r1, :]
        sdpn[:S_SLICE_RAW] = shared_down_w[:, r0:r1].T
        sg_c = np.ascontiguousarray(
            sgp.reshape(MS_, P, KH_, P).transpose(0, 3, 2, 1)
        ).reshape(MS_, P, KH_ * P).astype(BF16)
        su_c = np.ascontiguousarray(
            sup.reshape(MS_, P, KH_, P).transpose(0, 3, 2, 1)
        ).reshape(MS_, P, KH_ * P).astype(BF16)
        sd_c = np.ascontiguousarray(
            sdpn.reshape(MS_, P, H).transpose(1, 0, 2)
        ).reshape(P, MS_ * H).astype(BF16)
        in_maps.append({
            "xs": xsT, "xb": xb_c, "x8": x8_c,
            "wg": wg_c, "wu": wu_c, "wd": wd_c,
            "wg8": wg8_c, "wu8": wu8_c, "wd8": wd8_c,
            "sg": sg_c, "su": su_c, "sd": sd_c,
        })

    nc = _get_compiled(T, caps)
    results = run_bass_kernel_spmd(nc, in_maps, core_ids=list(range(N_CORES)))

    _LAST.clear()
    _LAST.update(nc=nc, in_maps=in_maps, results=results, caps=caps)

    # ---- host-side combine ----
    outT = np.zeros((H, T), np.float32)
    for c in range(N_CORES):
        outT += results.results[c]["ys"].astype(np.float32)
    for c in range(N_CORES):
        yev = results.results[c]["ye"].astype(np.float32)
        ye8v = results.results[c]["ye8"].astype(np.float32)
        for sl in range(EXP_PER_CORE):
            e = assign[c, sl]
            ib = idx_ec[(e, "bf")]
            if len(ib):
                we = comb[ib, e]
                b0 = base_b[sl]
                outT[:, ib] += yev[:, b0:b0 + len(ib)] * we[None, :]
            for cname in ("gu8", "f8"):
                ix = idx_ec[(e, cname)]
                if len(ix) == 0:
                    continue
                we = comb[ix, e] / OS
                b0 = base_8[(sl, cname)]
                outT[:, ix] += ye8v[:, b0:b0 + len(ix)] * we[None, :]

    return np.ascontiguousarray(outT.T).reshape(B, S, H).astype(np.float32)
